# revision 2
# baseline (speedup 1.0000x reference)
"""EMD loss kernel v2 for Trainium2 (8 NeuronCores, pure data parallel).

out[b] = sum_t (cumsum(x-y, axis=1)[b, t])^2 for x, y [131072, 256] f32.

v1 ran one 256-elem tensor_tensor_scan + one ACT Square+accum per row-block:
DVE 85us / ACT 95us busy vs the fixed ~91us HBM stream, so compute backlog
dragged a ~5-7us tail past the last DMA byte. v2 rebalances:

  - ONE contaminated scan per 8-block chunk (state = (x+s)-y runs across the
    row boundaries): scan cost drops 667 -> ~560ns/block (the scan's fixed
    overhead amortizes; its stream rate is ~2.13ns/elem regardless).
  - Carry correction is free: for block j>0 the true row cumsum is
    z_t - B_j with B_j = z[:, j*256-1] (the carry-in), and ACT's Square
    computes func(scale*in + bias) with a per-partition bias AP, so
    Square(scale=-1, bias=B_j) + accum_out gives sum((z_t - B_j)^2) in the
    same single instruction as before.
  - Block 0 of each chunk (B=0) moves off ACT entirely: DVE
    scalar_tensor_tensor (z*1)*z with accum_out does square+row-sum in one
    ~450ns op. Net: DVE ~82us, ACT ~84us, both under the DMA stream.
"""

import numpy as np

from concourse import bacc, bass, mybir
from concourse.bass_utils import run_bass_kernel_spmd
from concourse.tile import TileContext

N_CORES = 8
B = 131072
BINS = 256
ROWS = B // N_CORES  # 16384 rows per core
P = 128
N_BLK = ROWS // P  # 128 row-blocks per core
HEAD = [8] * 14
CHUNK_SLOT = 8
IO_BUFS = 8
TAIL = [8, 4, 2, 1, 1]
CHUNKS = HEAD + TAIL
assert sum(CHUNKS) == N_BLK

F32 = mybir.dt.float32
A = mybir.AluOpType
SQ = mybir.ActivationFunctionType.Square


def build_nc() -> bass.Bass:
    nc = bacc.Bacc()

    xy = nc.declare_dram_parameter("xy", [2, ROWS, BINS], F32, isOutput=False)
    out = nc.declare_dram_parameter("out", [ROWS], F32, isOutput=True)

    xyv = xy[:].rearrange("z (p n) d -> p z (n d)", p=P)
    ov = out[:].rearrange("(p n) -> p n", p=P)

    with (
        TileContext(nc) as tc,
        tc.tile_pool(name="io", bufs=IO_BUFS) as io_pool,
        tc.tile_pool(name="iotail", bufs=1) as tail_pool,
        tc.tile_pool(name="zp", bufs=3) as z_pool,
        tc.tile_pool(name="wv", bufs=2) as wv_pool,
        tc.tile_pool(name="res", bufs=1) as res_pool,
        tc.tile_pool(name="sq", bufs=8, space="PSUM") as sq_pool,
    ):
        out_sb = res_pool.tile([P, N_BLK], F32)

        # Warm the ACT Square table at t=0 so the table load overlaps the
        # first input DMAs.
        warm = res_pool.tile([P, 1], F32, tag="warm")
        warm2 = res_pool.tile([P, 1], F32, tag="warm2")
        nc.vector.memset(warm[:], 0)
        nc.scalar.activation(out=warm2[:], in_=warm[:], func=SQ)

        blk0 = 0
        for ci, tsz in enumerate(CHUNKS):
            if ci < len(HEAD):
                slot = CHUNK_SLOT
                xyt = io_pool.tile(
                    [P, 2 * slot * BINS], F32, tag="xyt", name=f"xyt{ci}"
                )
            else:
                slot = tsz
                xyt = tail_pool.tile(
                    [P, 2 * slot * BINS], F32, tag=f"tail{ci}", name=f"xyt{ci}"
                )
            xyt3 = xyt[:].rearrange("p (z f) -> p z f", z=2)[:, :, : tsz * BINS]
            lo, hi = blk0 * BINS, (blk0 + tsz) * BINS
            nc.sync.dma_start(out=xyt3, in_=xyv[:, :, lo:hi])

            F = tsz * BINS
            yoff = slot * BINS
            z = z_pool.tile([P, CHUNK_SLOT * BINS], F32, tag="z", name=f"z{ci}")
            # One scan across the whole chunk; state runs over row
            # boundaries (carry corrected per block below).
            nc.vector.tensor_tensor_scan(
                out=z[:, 0:F],
                data0=xyt[:, 0:F],
                data1=xyt[:, yoff : yoff + F],
                initial=0.0,
                op0=A.add,
                op1=A.subtract,
            )
            for j in range(tsz):
                col = blk0 + j
                seg = z[:, j * BINS : (j + 1) * BINS]
                if j == 0:
                    # carry-free: square+row-sum on DVE
                    w = wv_pool.tile([P, BINS], F32, tag="w", name=f"w{ci}")
                    nc.vector.scalar_tensor_tensor(
                        out=w[:],
                        in0=seg,
                        scalar=1.0,
                        in1=seg,
                        op0=A.mult,
                        op1=A.mult,
                        accum_out=out_sb[:, col : col + 1],
                    )
                else:
                    # sum((z - B)^2) = sum(Square(-z + B)); B = carry-in
                    bias = z[:, j * BINS - 1 : j * BINS]
                    sq = sq_pool.tile([P, BINS], F32)
                    nc.scalar.activation(
                        out=sq[:],
                        in_=seg,
                        func=SQ,
                        scale=-1.0,
                        bias=bias,
                        accum_out=out_sb[:, col : col + 1],
                    )
            blk0 += tsz
        nc.sync.dma_start(out=ov[:, :], in_=out_sb[:])
    nc.finalize()
    return nc


_NC = None


def _get_nc() -> bass.Bass:
    global _NC
    if _NC is None:
        _NC = build_nc()
    return _NC


def kernel(x: np.ndarray, y: np.ndarray) -> np.ndarray:
    assert x.shape == (B, BINS) and y.shape == (B, BINS), (x.shape, y.shape)
    x = np.ascontiguousarray(x, dtype=np.float32)
    y = np.ascontiguousarray(y, dtype=np.float32)
    in_maps = []
    for i in range(N_CORES):
        sl = slice(i * ROWS, (i + 1) * ROWS)
        in_maps.append({"xy": np.stack([x[sl], y[sl]])})
    res = run_bass_kernel_spmd(_get_nc(), in_maps, list(range(N_CORES)))
    return np.concatenate([m["out"] for m in res.results])
